# revision 7
# baseline (speedup 1.0000x reference)
"""BinaryDense TRN2 kernel: out = inputs @ binary_quantize(kernel) + bias.

binary_quantize: threshold t = mean(|kernel|) over the FULL [4096,4096] kernel;
qk = (|w| > t) ? 1.0 : 0.0.

Distribution (8 NeuronCores, tensor-parallel over units):
  - every core gets the full inputs, transposed on host to [D_IN, B] so the
    contraction dim lands on SBUF partitions (inputs_T is identical across cores),
  - core c gets kernel[:, c*512:(c+1)*512] and bias[c*512:(c+1)*512],
  - the threshold needs a global mean: each core reduces its own shard to a
    compensated (TwoSum) partial, a tiny AllGather shares all 8 partials, and
    every core finishes the exact same sum locally (bit-identical t everywhere),
  - core c computes out[:, c*512:(c+1)*512]; host concatenates shards on axis 1.

Matmul runs in float32r (full-rate fp32 path on the PE): stationary operand is a
[128,128] slice of inputs_T (self-loading f32r weights), moving operand is the
quantized [128,512] kernel tile, accumulating over the 32 k-tiles in PSUM.
"""

import numpy as np

import concourse.bacc as bacc
import concourse.mybir as mybir
import concourse.tile as tile
from concourse.bass_utils import run_bass_kernel_spmd
from concourse.masks import make_identity

B = 4096
DIN = 4096
UNITS = 4096
NCORES = 8
NU = UNITS // NCORES      # 512 units per core
P = 128
KT = DIN // P             # 32 contraction tiles
BGW = 1024                # batch-group width (one DMA slab)
NBG = B // BGW            # 4 batch groups
NB8 = BGW // P            # 8 PSUM accumulation groups per batch group

F32 = mybir.dt.float32
F32R = mybir.dt.float32r
X = mybir.AxisListType.X


def _twosum_tree(nc, sp, row, width, tag):
    """Pairwise-tree sum of row[1, width] with TwoSum error compensation.

    Returns (root, err_total): two [1,1] f32 tiles; the exact sum is
    root + err_total up to O(eps^2).
    """
    eacc = sp.tile([1, width // 2], F32, name=f"{tag}_eacc")
    nc.vector.memset(eacc[:], 0.0)
    cur = row
    w = width
    lvl = 0
    while w > 1:
        h = w // 2
        a = cur[:, 0:h]
        b = cur[:, h:w]
        s = sp.tile([1, h], F32, name=f"{tag}_s{lvl}")
        z = sp.tile([1, h], F32, name=f"{tag}_z{lvl}")
        t1 = sp.tile([1, h], F32, name=f"{tag}_t{lvl}")
        da = sp.tile([1, h], F32, name=f"{tag}_da{lvl}")
        db = sp.tile([1, h], F32, name=f"{tag}_db{lvl}")
        e = sp.tile([1, h], F32, name=f"{tag}_e{lvl}")
        nc.vector.tensor_add(s[:], a, b)        # s = a + b
        nc.vector.tensor_sub(z[:], s[:], a)     # z = s - a   (b')
        nc.vector.tensor_sub(t1[:], s[:], z[:])  # t1 = s - z  (a')
        nc.vector.tensor_sub(da[:], a, t1[:])   # da = a - a'
        nc.vector.tensor_sub(db[:], b, z[:])    # db = b - b'
        nc.vector.tensor_add(e[:], da[:], db[:])
        nc.vector.tensor_add(eacc[:, 0:h], eacc[:, 0:h], e[:])
        cur = s
        w = h
        lvl += 1
    etot = sp.tile([1, 1], F32, name=f"{tag}_etot")
    nc.vector.reduce_sum(etot[:], eacc[:], axis=X)
    return cur, etot


def _build():
    nc = bacc.Bacc("TRN2", target_bir_lowering=False, num_devices=NCORES)
    xT = nc.dram_tensor("xT", [DIN, B], F32R, kind="ExternalInput")
    ks = nc.dram_tensor("ks", [DIN, NU], F32, kind="ExternalInput")
    bs = nc.dram_tensor("bs", [NU], F32, kind="ExternalInput")
    out = nc.dram_tensor("out", [B, NU], F32, kind="ExternalOutput")
    dbg = nc.dram_tensor("dbg", [1, 32], F32, kind="ExternalOutput")

    with tile.TileContext(nc) as tc:
        with tc.tile_pool(name="qkp", bufs=1) as qkp, \
             tc.tile_pool(name="sp", bufs=1) as sp, \
             tc.tile_pool(name="slabp", bufs=6) as slabp, \
             tc.tile_pool(name="outp", bufs=4) as outp, \
             tc.tile_pool(name="psp", bufs=1, space="PSUM") as psp, \
             tc.tile_pool(name="dramp", bufs=1, space="DRAM") as dramp:

            # ---- kernel shard in, |w| row-sums per k-tile ----
            qk = []
            partials = sp.tile([P, KT], F32, name="partials")
            for k in range(KT):
                t_ = qkp.tile([P, NU], F32, tag=f"qk{k}", name=f"qk{k}")
                nc.sync.dma_start(t_[:], ks[k * P:(k + 1) * P, :])
                qk.append(t_)
            for k in range(KT):
                nc.vector.reduce_sum(partials[:, k:k + 1], qk[k][:], axis=X,
                                     apply_absolute_value=True)
            red1 = sp.tile([P, 1], F32, name="red1")
            nc.vector.reduce_sum(red1[:], partials[:], axis=X)

            # ---- partition dim -> one row via PE transpose (stays on-chip) ----
            ident = sp.tile([P, P], F32, name="ident")
            make_identity(nc, ident[:])
            pst = psp.tile([1, P], F32, tag="ps0", name="pst")
            nc.tensor.transpose(pst[:], red1[:], ident[:])
            rowa = sp.tile([1, P], F32, name="rowa")
            nc.vector.tensor_copy(rowa[:], pst[:])

            # ---- exact local sum, AllGather all 8 (sum, err) pairs ----
            s_c, e_c = _twosum_tree(nc, sp, rowa[:], P, "ts1")
            pair = sp.tile([1, 2], F32, name="pair")
            nc.vector.tensor_copy(pair[:, 0:1], s_c[:])
            nc.vector.tensor_copy(pair[:, 1:2], e_c[:])
            ccin = dramp.tile([1, 2], F32, name="ccin")
            nc.sync.dma_start(ccin[:], pair[:])
            ccout = dramp.tile([NCORES, 2], F32, name="ccout",
                               addr_space="Shared")
            nc.gpsimd.collective_compute(
                "AllGather", mybir.AluOpType.bypass,
                replica_groups=[list(range(NCORES))],
                ins=[ccin[:]], outs=[ccout[:]])
            row16 = sp.tile([1, 2 * NCORES], F32, name="row16")
            nc.sync.dma_start(row16[:], ccout[:].rearrange("a b -> (a b)"))

            # ---- global exact sum -> threshold t = S / 2^24 ----
            s_g, e_g = _twosum_tree(nc, sp, row16[:], 2 * NCORES, "ts2")
            stot = sp.tile([1, 1], F32, name="stot")
            nc.vector.tensor_add(stot[:], s_g[:], e_g[:])
            # broadcast S to all 128 partitions via a K=1 matmul, then the
            # exact power-of-two mean divide on evict
            ones128 = sp.tile([1, P], F32, name="ones128")
            nc.vector.memset(ones128[:], 1.0)
            psb = psp.tile([P, 1], F32, tag="ps1", name="psb")
            nc.tensor.matmul(psb[:], ones128[:], stot[:], start=True, stop=True)
            t128 = sp.tile([P, 1], F32, name="t128")
            nc.scalar.mul(t128[:], psb[:], 1.0 / (DIN * UNITS))

            # ---- debug dump of threshold pipeline ----
            nc.sync.dma_start(dbg[:, 0:16], row16[:])
            nc.sync.dma_start(dbg[:, 16:17], stot[:])
            nc.sync.dma_start(dbg[:, 17:18], t128[0:1, :])
            nc.sync.dma_start(dbg[:, 18:19], s_c[:])
            nc.sync.dma_start(dbg[:, 19:20], e_c[:])
            nc.sync.dma_start(dbg[:, 20:28], rowa[:, 0:8])
            nc.sync.dma_start(dbg[:, 28:29], red1[0:1, :])
            nc.sync.dma_start(dbg[:, 29:30], t128[0:1, :])
            nc.sync.dma_start(dbg[:, 30:31], partials[0:1, 0:1])
            nc.sync.dma_start(dbg[:, 31:32], partials[0:1, 1:2])

            # ---- quantize: qk2 = (w > t) + (-w > t)  == (|w| > t) ----
            # (both compares run on the raw f32 weights, so the decision is
            # bit-exact; the f32r-typed outputs only ever hold 0.0/1.0)
            qk2 = []
            for k in range(KT):
                q_ = qkp.tile([P, NU], F32R, tag=f"qk2_{k}", name=f"qk2_{k}")
                nc.vector.tensor_scalar(q_[:], qk[k][:], t128[:, 0:1], None,
                                        op0=mybir.AluOpType.is_gt)
                c2 = sp.tile([P, NU], F32R, tag="c2tmp", name=f"c2_{k}", bufs=3)
                nc.vector.tensor_scalar(c2[:], qk[k][:], -1.0, t128[:, 0:1],
                                        op0=mybir.AluOpType.mult,
                                        op1=mybir.AluOpType.is_gt)
                nc.vector.tensor_add(q_[:], q_[:], c2[:])
                qk2.append(q_)

            # ---- bias broadcast to all partitions ----
            biasf = sp.tile([P, NU], F32, name="biasf")
            nc.sync.dma_start(biasf[:], bs[:].rearrange("(a b) -> a b", a=1).to_broadcast((P, NU)))

            # ---- main matmul: out[b,u] = sum_k xT[k,b] * qk[k,u] ----
            for bg in range(NBG):
                ps = [psp.tile([P, NU], F32, tag=f"ps{b}", name=f"ps_{bg}_{b}")
                      for b in range(NB8)]
                for k in range(KT):
                    slab = slabp.tile([P, BGW], F32R, tag="slab",
                                      name=f"slab_{bg}_{k}")
                    nc.sync.dma_start(
                        slab[:], xT[k * P:(k + 1) * P, bg * BGW:(bg + 1) * BGW])
                    for b in range(NB8):
                        nc.tensor.matmul(
                            ps[b][:],
                            slab[:, b * P:(b + 1) * P],
                            qk2[k][:],
                            start=(k == 0), stop=(k == KT - 1))
                for b in range(NB8):
                    ob = outp.tile([P, NU], F32, tag="ob", name=f"ob_{bg}_{b}")
                    nc.vector.tensor_add(ob[:], ps[b][:], biasf[:])
                    row = (bg * NB8 + b) * P
                    nc.sync.dma_start(out[row:row + P, :], ob[:])
    nc.compile()
    return nc


_NC = None


def _get_nc():
    global _NC
    if _NC is None:
        _NC = _build()
    return _NC


def run(inputs, kernel, bias, trace=False, trace_cores=None):
    nc = _get_nc()
    x = np.ascontiguousarray(np.asarray(inputs, dtype=np.float32).T)
    w = np.asarray(kernel, dtype=np.float32)
    b = np.asarray(bias, dtype=np.float32)
    in_maps = [{
        "xT": x,
        "ks": np.ascontiguousarray(w[:, c * NU:(c + 1) * NU]),
        "bs": np.ascontiguousarray(b[c * NU:(c + 1) * NU]),
    } for c in range(NCORES)]
    res = run_bass_kernel_spmd(nc, in_maps, core_ids=list(range(NCORES)),
                               trace=trace, trace_cores=trace_cores)
    full = np.concatenate([r["out"] for r in res.results], axis=1)
    return full, res


def kernel(inputs, kernel, bias):
    full, _ = run(inputs, kernel, bias)
    return full
